# revision 2
# baseline (speedup 1.0000x reference)
"""Trainium2 Bass kernel: separable box filter (radius 4) on (8,3,1024,1024) fp32.

Equivalent to the reference:
    box(x) = diff(cumsum(diff(cumsum(x, H), H), W), W)    # truncated 9x9 box sum

Strategy (pure data parallel over the 24 (n,c) slices, 3 per core):
  - W pass: DVE cumsum (tensor_tensor_scan) along the free axis into a padded
    buffer Cp = [0 x5 | cumsum_w x | replicate(C[1023]) x4], so that
    boxW(x)[w] = Cp[w+9] - Cp[w] for every w with truncated-window semantics.
  - H pass + W diff fused on the PE: overlapping input tiles of 128 rows
    produce 120 output rows each via a banded weight matrix
    W[m, k] = 1 iff m <= k <= m+8 (out row m consumes in rows m..m+8 of the
    tile, which covers global rows 120t-4 .. 120t+123).  The W diff is folded
    in by accumulating  (+W) @ Cp[:, w0+9:...]  and  (-W) @ Cp[:, w0:...]
    into the same PSUM bank.
  - ACT copies PSUM -> SBUF, DMA out.
"""

import numpy as np

H = 1024
W = 1024
R = 4
N_CORES = 8
SLICES_PER_CORE = 3  # 8*3 = 24 (n,c) slices / 8 cores
TILE_OUT = 120  # output rows per PE tile (128 input rows - 2*R)
N_TILES = 9  # ceil(1024 / 120); last tile emits 64 rows
CP_W = W + 9  # 5 left zeros + 1024 cumsum + 4 replicated
F32 = None  # set lazily (mybir.dt.float32)

_COMPILED = {}


def _band_weights():
    """lhsT for the H-pass band matmul: [K=128, M=120], lhsT[k, m] = W[m, k],
    W[m, k] = 1 iff m <= k <= m+8."""
    k = np.arange(128)[:, None]
    m = np.arange(TILE_OUT)[None, :]
    pos = ((m <= k) & (k <= m + 2 * R)).astype(np.float32)
    return pos, -pos


def _build():
    from contextlib import ExitStack  # noqa: F401
    from concourse import bacc, bass, mybir
    from concourse.tile import TileContext

    f32 = mybir.dt.float32
    nc = bacc.Bacc("TRN2", target_bir_lowering=False, debug=False,
                   num_devices=N_CORES)

    x = nc.dram_tensor("x", (SLICES_PER_CORE, H, W), f32,
                       kind="ExternalInput").ap()
    wp = nc.dram_tensor("wp", (128, TILE_OUT), f32, kind="ExternalInput").ap()
    wn = nc.dram_tensor("wn", (128, TILE_OUT), f32, kind="ExternalInput").ap()
    out = nc.dram_tensor("out", (SLICES_PER_CORE, H, W), f32,
                         kind="ExternalOutput").ap()

    add = mybir.AluOpType.add
    bypass = mybir.AluOpType.bypass
    act_copy = mybir.ActivationFunctionType.Copy

    with TileContext(nc) as tc:
        with tc.tile_pool(name="wts", bufs=1) as wpool, \
             tc.tile_pool(name="xin", bufs=4) as xpool, \
             tc.tile_pool(name="cp", bufs=4) as cpool, \
             tc.tile_pool(name="outp", bufs=4) as opool, \
             tc.tile_pool(name="ps", bufs=8, space="PSUM") as pspool:
            wp_t = wpool.tile([128, TILE_OUT], f32)
            wn_t = wpool.tile([128, TILE_OUT], f32)
            nc.sync.dma_start(wp_t[:], wp[:])
            nc.sync.dma_start(wn_t[:], wn[:])

            for s in range(SLICES_PER_CORE):
                for t in range(N_TILES):
                    r0 = max(0, TILE_OUT * t - R)
                    r1 = min(H, TILE_OUT * t + 128 - R)
                    p0 = r0 - (TILE_OUT * t - R)
                    cnt = r1 - r0
                    m = min(TILE_OUT, H - TILE_OUT * t)  # output rows

                    xt = xpool.tile([128, W], f32)
                    # Engine accesses need 32-aligned partition bases: memset
                    # an aligned superset first, DMA overwrites the overlap.
                    if p0 > 0:
                        a = (p0 + 31) // 32 * 32
                        nc.gpsimd.memset(xt[0:a, :], 0.0)
                    if p0 + cnt < 128:
                        a = (p0 + cnt) // 32 * 32
                        nc.gpsimd.memset(xt[a:128, :], 0.0)
                    nc.sync.dma_start(xt[p0:p0 + cnt, :], x[s, r0:r1, :])

                    cp = cpool.tile([128, CP_W], f32)
                    nc.gpsimd.memset(cp[:, 0:2 * R + 1 - R], 0.0)  # cols 0..4
                    nc.vector.tensor_tensor_scan(
                        cp[:, R + 1:R + 1 + W], xt[:, :], xt[:, :], 0.0,
                        add, bypass)
                    nc.vector.tensor_copy(
                        cp[:, R + 1 + W:CP_W],
                        cp[:, R + W:R + 1 + W].broadcast_to([128, R]))

                    ot = opool.tile([TILE_OUT, W], f32)
                    for hf in range(2):
                        w0 = 512 * hf
                        ps = pspool.tile([TILE_OUT, 512], f32)
                        nc.tensor.matmul(ps[:], wp_t[:],
                                         cp[:, w0 + 2 * R + 1:w0 + 521],
                                         start=True, stop=False)
                        nc.tensor.matmul(ps[:], wn_t[:], cp[:, w0:w0 + 512],
                                         start=False, stop=True)
                        nc.scalar.activation(ot[0:m, w0:w0 + 512], ps[0:m, :],
                                             act_copy)
                    nc.sync.dma_start(out[s, TILE_OUT * t:TILE_OUT * t + m, :],
                                      ot[0:m, :])

    nc.compile()
    return nc


def _get_nc():
    if "nc" not in _COMPILED:
        _COMPILED["nc"] = _build()
    return _COMPILED["nc"]


def kernel(x: np.ndarray) -> np.ndarray:
    from concourse.bass_utils import run_bass_kernel_spmd

    nc = _get_nc()
    xf = np.ascontiguousarray(np.asarray(x, dtype=np.float32)).reshape(
        N_CORES * SLICES_PER_CORE, H, W)
    wp_np, wn_np = _band_weights()
    in_maps = []
    for c in range(N_CORES):
        in_maps.append({
            "x": xf[c * SLICES_PER_CORE:(c + 1) * SLICES_PER_CORE],
            "wp": wp_np,
            "wn": wn_np,
        })
    res = run_bass_kernel_spmd(nc, in_maps, core_ids=list(range(N_CORES)))
    outs = [res.results[c]["out"] for c in range(N_CORES)]
    full = np.concatenate(outs, axis=0).reshape(8, 3, H, W)
    return full
